# revision 3
# baseline (speedup 1.0000x reference)
"""Trainium2 Bass kernel for nn_CoAttentionLayer2 (dense_transformer).

Sharding: pure data parallel - batch B=8 mapped 1:1 onto 8 NeuronCores.
Each core runs the full co-attention layer for one batch element; no
collectives. Weights are replicated (cast to fp16 on host).

Per-core pipeline (one batch element, Nq=Nk=1024, D=512, 8 heads x 64):
  1. LayerNorm in token-major fp32 (bn_stats/bn_aggr; gamma/beta folded
     into the weights on the host), xhat emitted fp16.
  2. xhat -> feature-major xhatT via DMA xbar transpose (no PE, no PSUM).
  3. Projections all-fp16 (1 cyc/row on PE): Q^T/K^T feature-major with
     the bias fused into the DVE PSUM->SBUF copyback (per-partition
     scalar); V token-major into an augmented [keys, 128]-per-head tile
     whose last 64 columns are ones -> attn@v broadcasts the softmax
     row-sums into PSUM partitions 64:128.
  4. Attention in query chunks of 512 (pairs inner): per (pair, kt):
     dots^T via two concurrent half-array matmuls, exp on ScalarE
     (scale=1/8 folded; no max subtraction - logits are O(5)), attn@v
     per head into a [128, 512] PSUM where rows 64:128 are the row-sums.
     Normalize = reciprocal_approx_fast + one fused multiply on DVE.
  5. Output projection per chunk (overlaps the next chunk's attention),
     PSUM->SBUF copyback, one DMA out per chunk.
"""

import numpy as np

import concourse.bass as bass
import concourse.mybir as mybir
import concourse.tile as tile
from concourse import bacc
from concourse.bass_utils import run_bass_kernel_spmd

P = 128
B = 8
N = 1024  # tokens (queries == keys)
D = 512  # model dim
HEADS = 8
DH = 64
INNER = 512
SCALE = DH**-0.5
EPS = 1e-5
F32 = mybir.dt.float32
F16 = mybir.dt.float16

KO = D // P  # 4 contraction tiles
JT = INNER // P  # 4 output-feature tiles (head pairs)
TT = N // P  # 8 token tiles
IC = 2  # query chunks
NQC = N // IC  # 512
KTT = TT  # 8 key tiles per chunk-pair


def _build_nc():
    nc = bacc.Bacc(
        "TRN2",
        target_bir_lowering=False,
        debug=False,
        num_devices=B,
    )

    xq_d = nc.declare_dram_parameter("xq", [N, D], F32, isOutput=False)
    xkv_d = nc.declare_dram_parameter("xkv", [N, D], F32, isOutput=False)
    wq_d = nc.declare_dram_parameter("wq", [D, INNER], F16, isOutput=False)
    wk_d = nc.declare_dram_parameter("wk", [D, INNER], F16, isOutput=False)
    wv_d = nc.declare_dram_parameter("wv", [D, INNER], F16, isOutput=False)
    wo_d = nc.declare_dram_parameter("wo", [INNER, D], F16, isOutput=False)
    bq_d = nc.declare_dram_parameter("bq", [INNER], F32, isOutput=False)
    bk_d = nc.declare_dram_parameter("bk", [INNER], F32, isOutput=False)
    bv_d = nc.declare_dram_parameter("bv", [INNER], F32, isOutput=False)
    out_d = nc.declare_dram_parameter("out", [N, D], F32, isOutput=True)

    with tile.TileContext(nc) as tc:
        with (
            tc.tile_pool(name="singles", bufs=1) as singles,
            tc.tile_pool(name="big", bufs=1) as big,
            tc.tile_pool(name="work", bufs=3) as work,
            tc.tile_pool(name="ps", bufs=2, space="PSUM") as ps,
        ):
            # ---- persistent SBUF ----
            xt_kv = singles.tile([P, TT, D], F32)  # token-major inputs
            xt_q = singles.tile([P, TT, D], F32)
            xhatT_kv = singles.tile([P, TT, KO, P], F16)  # [c%128, tt, c//128, tok%128]
            xhatT_q = singles.tile([P, TT, KO, P], F16)
            QT = big.tile([P, JT, N], F16)  # [j%128, j//128, token]
            KT = big.tile([P, JT, N], F16)
            # V augmented: cols 0:64 = V_h + bv_h, cols 64:128 = ones
            Vg = big.tile([P, TT, HEADS, 2 * DH], F16)
            outT = big.tile([P, KO, N], F16)  # [c%128, c//128, token]
            ot = singles.tile([P, IC, KO, D], F32)  # o_proj staging per chunk

            wq_sb = singles.tile([P, KO, INNER], F16)
            wk_sb = singles.tile([P, KO, INNER], F16)
            wv_sb = singles.tile([P, KO, INNER], F16)
            wo_sb = singles.tile([P, KO, D], F16)
            bq_sb = singles.tile([P, JT], F32)
            bk_sb = singles.tile([P, JT], F32)
            bvB = singles.tile([P, INNER], F32)
            eps_sb = singles.tile([P, 1], F32)

            # ---- input DMAs first so HBM reads start immediately ----
            nc.sync.dma_start(
                out=xt_kv[:, 0:4, :],
                in_=xkv_d[0:512, :].rearrange("(t p) d -> p t d", p=P),
            )
            nc.sync.dma_start(
                out=xt_q[:, 0:4, :],
                in_=xq_d[0:512, :].rearrange("(t p) d -> p t d", p=P),
            )
            nc.sync.dma_start(
                out=xt_kv[:, 4:8, :],
                in_=xkv_d[512:1024, :].rearrange("(t p) d -> p t d", p=P),
            )
            nc.sync.dma_start(
                out=xt_q[:, 4:8, :],
                in_=xq_d[512:1024, :].rearrange("(t p) d -> p t d", p=P),
            )
            # weights on the act HWDGE queue (ACT is idle until exp starts)
            nc.scalar.dma_start(out=wk_sb[:], in_=wk_d.rearrange("(ko p) j -> p ko j", p=P))
            nc.scalar.dma_start(out=wq_sb[:], in_=wq_d.rearrange("(ko p) j -> p ko j", p=P))
            nc.scalar.dma_start(out=wv_sb[:], in_=wv_d.rearrange("(ko p) j -> p ko j", p=P))
            nc.scalar.dma_start(out=wo_sb[:], in_=wo_d.rearrange("(co p) j -> p co j", p=P))
            nc.scalar.dma_start(out=bq_sb[:], in_=bq_d.rearrange("(t p) -> p t", p=P))
            nc.scalar.dma_start(out=bk_sb[:], in_=bk_d.rearrange("(t p) -> p t", p=P))
            bv_ap = bv_d.ap()
            bv_bcast = bass.AP(
                tensor=bv_ap.tensor, offset=bv_ap.offset, ap=[[0, P], [1, INNER]]
            )
            nc.scalar.dma_start(out=bvB[:], in_=bv_bcast)

            nc.vector.memset(eps_sb, EPS)
            nc.vector.memset(Vg[:, :, :, DH : 2 * DH], 1.0)

            # ---- stage emitters ----
            def ln_transpose(xt_big, xhatT, tt, eng):
                """LayerNorm token tile tt, then DMA-xbar-transpose into xhatT."""
                xt = xt_big[:, tt, :]
                stats = work.tile([P, 6], F32, tag="ln_stats")
                nc.vector.bn_stats(out=stats[:], in_=xt)
                mv = work.tile([P, 2], F32, tag="ln_mv")
                nc.vector.bn_aggr(out=mv[:], in_=stats[:])
                std = work.tile([P, 1], F32, tag="ln_std")
                nc.scalar.activation(
                    out=std[:],
                    in_=mv[:, 1:2],
                    func=mybir.ActivationFunctionType.Sqrt,
                    bias=eps_sb[:],
                    scale=1.0,
                )
                rstd = work.tile([P, 1], F32, tag="ln_rstd")
                nc.vector.reciprocal(out=rstd[:], in_=std[:])
                nmr = work.tile([P, 1], F32, tag="ln_nmr")
                nc.vector.tensor_tensor(
                    out=nmr[:], in0=mv[:, 0:1], in1=rstd[:], op=mybir.AluOpType.mult
                )
                xhat = work.tile([P, D], F16, tag="xhat", bufs=4)
                eng.tensor_scalar(
                    out=xhat[:],
                    in0=xt,
                    scalar1=rstd[:],
                    scalar2=nmr[:],
                    op0=mybir.AluOpType.mult,
                    op1=mybir.AluOpType.subtract,
                )
                # out[p, ko, t] = xhat[t, ko*128 + p]; dest is contiguous per
                # partition so the xbar fast path is safe
                dq = nc.sync if tt % 2 == 0 else nc.scalar
                dq.dma_start_transpose(out=xhatT[:, tt, :, :], in_=xhat[:])

            def qk_proj(w_sb, b_sb, xhatT, dstT, j, ic):
                """Feature tile j (head pair j), query/key half ic -> dstT."""
                pm = ps.tile([P, NQC], F32, tag="sm", name="qkpm")
                for ko in range(KO):
                    nc.tensor.matmul(
                        pm[:],
                        w_sb[:, ko, j * P : (j + 1) * P],
                        xhatT[:, 4 * ic : 4 * ic + 4, ko, :],
                        start=(ko == 0),
                        stop=(ko == KO - 1),
                    )
                # PSUM->SBUF copyback with the bias fused (per-partition scalar)
                nc.vector.tensor_scalar(
                    out=dstT[:, j, ic * NQC : (ic + 1) * NQC],
                    in0=pm[:],
                    scalar1=b_sb[:, j : j + 1],
                    scalar2=None,
                    op0=mybir.AluOpType.add,
                )

            def v_proj(tt):
                """V projection (token-major) into the augmented V tile."""
                pm = ps.tile([P, INNER], F32, tag="sm", name="vpm")
                for ko in range(KO):
                    nc.tensor.matmul(
                        pm[:],
                        xhatT_kv[:, tt, ko, :],
                        wv_sb[:, ko, :],
                        start=(ko == 0),
                        stop=(ko == KO - 1),
                    )
                nc.vector.tensor_tensor(
                    out=Vg[:, tt, :, 0:DH],
                    in0=pm[:].rearrange("p (h d) -> p h d", d=DH),
                    in1=bvB.rearrange("p (h d) -> p h d", d=DH),
                    op=mybir.AluOpType.add,
                )

            LAG = 2  # attn@v trails exp by this many kt steps

            def attention_pair(hq, ic):
                """Heads 2hq, 2hq+1 for query chunk ic."""
                h0, h1 = 2 * hq, 2 * hq + 1
                po0 = ps.tile([P, NQC], F32, tag="po", name="po0")
                po1 = ps.tile([P, NQC], F32, tag="po", name="po1")
                exs = []
                for kt in range(KTT):
                    pd = ps.tile([P, N], F32, tag="big", name="pd")
                    for hh in range(2):
                        nc.tensor.matmul(
                            pd[:, hh * NQC : (hh + 1) * NQC],
                            KT[hh * DH : (hh + 1) * DH, hq, kt * P : (kt + 1) * P],
                            QT[hh * DH : (hh + 1) * DH, hq, ic * NQC : (ic + 1) * NQC],
                            start=True,
                            stop=True,
                            tile_position=(hh * DH, 0),
                        )
                    ex = work.tile([P, N], F16, tag="expT", bufs=LAG + 3)
                    nc.scalar.activation(
                        out=ex[:],
                        in_=pd[:],
                        func=mybir.ActivationFunctionType.Exp,
                        scale=SCALE,
                    )
                    exs.append(ex)
                    if kt >= LAG:
                        _attnv(po0, po1, h0, h1, kt - LAG, exs[kt - LAG])
                for kt in range(KTT - LAG, KTT):
                    _attnv(po0, po1, h0, h1, kt, exs[kt])
                for po, hh in ((po0, 0), (po1, 1)):
                    _normalize(po, hq, hh, ic)

            def _attnv(po0, po1, h0, h1, kt, ex):
                for po, h, hh in ((po0, h0, 0), (po1, h1, 1)):
                    nc.tensor.matmul(
                        po[:],
                        Vg[:, kt, h, :],
                        ex[:, hh * NQC : (hh + 1) * NQC],
                        start=(kt == 0),
                        stop=(kt == KTT - 1),
                    )

            def _normalize(po, hq, hh, ic):
                # po rows 0:64 = attn@V, rows 64:128 = row-sums (broadcast by
                # the ones block of Vg); one recip + one fused multiply
                rb = work.tile([DH, NQC], F32, tag="recB")
                nc.vector.reciprocal_approx_fast(out=rb[:], in_=po[DH:P, :])
                nc.vector.tensor_tensor(
                    out=outT[hh * DH : (hh + 1) * DH, hq, ic * NQC : (ic + 1) * NQC],
                    in0=po[0:DH, :],
                    in1=rb[:],
                    op=mybir.AluOpType.mult,
                )

            def o_proj(tt, ic, eng):
                pm = ps.tile([P, D], F32, tag="sm", name="opm")
                for co in range(KO):
                    nc.tensor.matmul(
                        pm[:],
                        outT[:, co, tt * P : (tt + 1) * P],
                        wo_sb[:, co, :],
                        start=(co == 0),
                        stop=(co == KO - 1),
                    )
                if eng is nc.scalar:
                    nc.scalar.copy(out=ot[:, ic, tt % 4, :], in_=pm[:])
                else:
                    nc.vector.tensor_copy(out=ot[:, ic, tt % 4, :], in_=pm[:])

            # ---- emission order (priority hints for the Tile scheduler) ----
            for tt in range(4):
                ln_transpose(xt_kv, xhatT_kv, tt, nc.gpsimd)
            qk_proj(wk_sb, bk_sb, xhatT_kv, KT, 0, 0)
            for tt in range(4):
                ln_transpose(xt_q, xhatT_q, tt, nc.vector)
            qk_proj(wq_sb, bq_sb, xhatT_q, QT, 0, 0)
            for tt in range(4, 8):
                ln_transpose(xt_kv, xhatT_kv, tt, nc.gpsimd)
            qk_proj(wk_sb, bk_sb, xhatT_kv, KT, 0, 1)
            v_proj(0)
            v_proj(1)
            attention_pair(0, 0)
            for tt in range(2, 8):
                v_proj(tt)
            for tt in range(4, 8):
                ln_transpose(xt_q, xhatT_q, tt, nc.vector)
            qk_proj(wq_sb, bq_sb, xhatT_q, QT, 0, 1)
            for j in range(1, JT):
                qk_proj(wk_sb, bk_sb, xhatT_kv, KT, j, 0)
                qk_proj(wk_sb, bk_sb, xhatT_kv, KT, j, 1)
            qk_proj(wq_sb, bq_sb, xhatT_q, QT, 1, 0)
            attention_pair(1, 0)
            qk_proj(wq_sb, bq_sb, xhatT_q, QT, 2, 0)
            attention_pair(2, 0)
            qk_proj(wq_sb, bq_sb, xhatT_q, QT, 3, 0)
            attention_pair(3, 0)
            for j in range(JT):
                qk_proj(wq_sb, bq_sb, xhatT_q, QT, j, 1)
            # chunk-0 output projection overlaps chunk-1 attention
            for tt in range(4):
                o_proj(tt, 0, nc.vector)
            nc.sync.dma_start(
                out=out_d[0:NQC, :].rearrange("(t p) d -> p t d", p=P),
                in_=ot[:, 0, :, :],
            )
            attention_pair(0, 1)
            attention_pair(1, 1)
            attention_pair(2, 1)
            attention_pair(3, 1)
            for tt in range(4, 8):
                o_proj(tt, 1, nc.scalar)
            nc.sync.dma_start(
                out=out_d[NQC:N, :].rearrange("(t p) d -> p t d", p=P),
                in_=ot[:, 1, :, :],
            )

    nc.compile()
    return nc


_NC_CACHE = {}


def _get_nc():
    if "nc" not in _NC_CACHE:
        _NC_CACHE["nc"] = _build_nc()
    return _NC_CACHE["nc"]


def _prep_in_maps(query, keyvalue, Wq, Wkv, Wo, gamma, beta):
    query = np.ascontiguousarray(query, dtype=np.float32)
    keyvalue = np.ascontiguousarray(keyvalue, dtype=np.float32)
    Wq = np.asarray(Wq, dtype=np.float32)
    Wkv = np.asarray(Wkv, dtype=np.float32)
    Wo = np.ascontiguousarray(Wo, dtype=np.float32)
    gamma = np.asarray(gamma, dtype=np.float32)
    beta = np.asarray(beta, dtype=np.float32)

    # fold LN affine into the projections: (xhat*g + b) @ W = xhat @ (g[:,None]*W) + b @ W
    wq_eff = np.ascontiguousarray((gamma[:, None] * Wq).astype(np.float16))
    wkv_eff = gamma[:, None] * Wkv
    bq = np.ascontiguousarray(beta @ Wq)
    bkv = beta @ Wkv
    wk_eff = np.ascontiguousarray(wkv_eff[:, :INNER].astype(np.float16))
    wv_eff = np.ascontiguousarray(wkv_eff[:, INNER:].astype(np.float16))
    bk = np.ascontiguousarray(bkv[:INNER])
    bv = np.ascontiguousarray(bkv[INNER:])
    wo_eff = np.ascontiguousarray(Wo.astype(np.float16))

    return [
        dict(
            xq=np.ascontiguousarray(query[b]),
            xkv=np.ascontiguousarray(keyvalue[b]),
            wq=wq_eff,
            wk=wk_eff,
            wv=wv_eff,
            wo=wo_eff,
            bq=bq,
            bk=bk,
            bv=bv,
        )
        for b in range(B)
    ]


def run_sharded(inputs, **spmd_kwargs):
    """Run the SPMD kernel; returns (stacked output [B, N, D], BassKernelResults)."""
    nc = _get_nc()
    in_maps = _prep_in_maps(**inputs)
    r = run_bass_kernel_spmd(nc, in_maps, core_ids=list(range(B)), **spmd_kwargs)
    out = np.stack([r.results[b]["out"] for b in range(B)], axis=0)
    return out, r


def kernel(query, keyvalue, Wq, Wkv, Wo, gamma, beta):
    out, _ = run_sharded(
        dict(query=query, keyvalue=keyvalue, Wq=Wq, Wkv=Wkv, Wo=Wo, gamma=gamma, beta=beta)
    )
    return out
